# revision 2
# baseline (speedup 1.0000x reference)
"""DSimilarity.gradgrad force-force covariance block on 8 Trainium2 cores.

out[m*3+a, n*3+b] = sum_{i,j} u1[i,a]*u2[j,b]*gg[i,j]*[i1[i]==m]*[i2[j]==n]
with gg[i,j] = (c - c^2 diff^2) * exp(-0.5 c diff^2), diff = d1[i]-d2[j], c=1/l^2.

Strategy: out = S1T.T @ gg @ S2 with sparse scatter matrices densified after
sorting pairs by atom index. Axis-2 (j) is sorted by i2 and sharded 1/8 per
core -> each core produces a contiguous strip of output columns (overlap-add
at boundary atoms on the host). Axis-1 (i) is sorted by i1 and grouped into
42-atom blocks (126 output rows <= 128 partitions) so stage B is
block-diagonal. gg is computed on-chip (ACT Square with per-partition bias,
ACT Exp, one fused DVE scalar_tensor_tensor); it never touches HBM.
"""

import math
import sys
import types

import numpy as np

NCORES = 8
ABLK = 42  # atoms per stage-B row block (126 rows)
SCW = 1536  # elementwise super-chunk width (free-dim)

TRACE = False  # test.py sets True to capture an NTFF profile
LAST_RESULTS = None  # BassKernelResults of the last run (for test.py)

_PROGRAM_CACHE = {}


def _install_ntff_hook():
    try:
        from antenv.axon_hooks import get_axon_ntff_profile_hook  # noqa: F401
        return
    except ImportError:
        pass
    try:
        from trn_agent_boot.trn_boot import _ntff_profile_via_ctypes
        import antenv
        hook = _ntff_profile_via_ctypes('/opt/axon/libaxon_pjrt.so')
        mod = types.ModuleType("antenv.axon_hooks")
        mod._hook = hook
        mod.get_axon_ntff_profile_hook = lambda: mod._hook
        mod.set_axon_ntff_profile_hook = lambda h: setattr(mod, "_hook", h)
        antenv.axon_hooks = mod
        sys.modules["antenv.axon_hooks"] = mod
    except Exception:
        pass


def _build_program(IPAD, NJ2, W3, block_chunks, sqrtc, lnc):
    """Compile the per-core Bass program. All 8 cores run this same program on
    different data. Shapes/constants are baked in.

    block_chunks: tuple of (chunk_start, nchunks) per 42-atom row block, in
    units of 128-pair i-chunks within the packed IPAD axis.
    """
    import concourse.bacc as bacc
    import concourse.tile as tile
    import concourse.mybir as mybir

    F32 = mybir.dt.float32
    Alu = mybir.AluOpType
    Act = mybir.ActivationFunctionType

    NIC = IPAD // 128
    NB = len(block_chunks)

    # output-column chunks of <=512 (PSUM bank limit); normally one chunk
    col_chunks = []
    c0 = 0
    while c0 < W3:
        col_chunks.append((c0, min(512, W3 - c0)))
        c0 += 512

    nc = bacc.Bacc("TRN2", target_bir_lowering=False, debug=False)
    d1_h = nc.dram_tensor("d1p", [IPAD], F32, kind="ExternalInput")
    d2_h = nc.dram_tensor("d2col", [128, NJ2], F32, kind="ExternalInput")
    s2_h = nc.dram_tensor("s2", [NJ2, 128, W3], F32, kind="ExternalInput")
    s1_h = nc.dram_tensor("s1t", [NIC, 128, 126], F32, kind="ExternalInput")
    out_h = nc.dram_tensor("out", [NB * 126, W3], F32, kind="ExternalOutput")

    with tile.TileContext(nc) as tc:
        with (
            tc.tile_pool(name="const", bufs=1) as cpool,
            tc.tile_pool(name="scratch", bufs=3) as spool,
            tc.tile_pool(name="hps", bufs=4, space="PSUM") as hpool,
            tc.tile_pool(name="ops", bufs=2, space="PSUM") as opool,
            tc.tile_pool(name="osb", bufs=3) as obpool,
        ):
            d1_rep = cpool.tile([128, IPAD], F32)
            nsl = 4
            for k in range(nsl):
                a = (IPAD // nsl // 128) * 128 * k
                b = IPAD if k == nsl - 1 else (IPAD // nsl // 128) * 128 * (k + 1)
                nc.sync.dma_start(out=d1_rep[:, a:b],
                                  in_=d1_h[a:b].partition_broadcast(128))
            d2c = cpool.tile([128, NJ2], F32)
            nc.sync.dma_start(out=d2c[:, :], in_=d2_h[:, :])
            s2_sb = cpool.tile([128, NJ2, W3], F32)
            for q in range(NJ2):
                nc.sync.dma_start(out=s2_sb[:, q, :], in_=s2_h[q, :, :])
            s1_sb = cpool.tile([128, NIC, 126], F32)
            s1_view = s1_h.ap().rearrange("t p m -> p t m")
            for k in range(nsl):
                a = NIC * k // nsl
                b = NIC * (k + 1) // nsl
                if a < b:
                    nc.sync.dma_start(out=s1_sb[:, a:b, :], in_=s1_view[:, a:b, :])

            gg = cpool.tile([128, NJ2, IPAD], F32)
            h_sb = cpool.tile([128, NIC, W3], F32)

            # super-chunked elementwise + stage A + stage B, in dataflow order
            n_sc = (IPAD + SCW - 1) // SCW
            ew_k = 0  # round-robin: square on ACT vs DVE (load balance)
            cp_k = 0  # round-robin PSUM->SBUF copy engine
            blocks_emitted = [False] * NB
            for sc in range(n_sc):
                a = sc * SCW
                b = min(IPAD, a + SCW)
                for q in range(NJ2):
                    sq = spool.tile([128, SCW], F32, tag="sq")
                    ex = spool.tile([128, SCW], F32, tag="ex")
                    w = b - a
                    if ew_k % 2 == 0:
                        # sq = (sqrt(c)*d2[p] - sqrt(c)*d1)^2 = c*diff^2 on ACT
                        nc.scalar.activation(sq[:, :w], d1_rep[:, a:b], Act.Square,
                                             bias=d2c[:, q:q + 1], scale=-sqrtc)
                    else:
                        # same on DVE: dp = -sqrt(c)*d1 + sqrt(c)*d2[p]; sq = dp*dp
                        dp = spool.tile([128, SCW], F32, tag="dp")
                        nc.vector.tensor_scalar(dp[:, :w], d1_rep[:, a:b],
                                                -sqrtc, d2c[:, q:q + 1],
                                                op0=Alu.mult, op1=Alu.add)
                        nc.vector.tensor_tensor(sq[:, :w], dp[:, :w], dp[:, :w],
                                                op=Alu.mult)
                    ew_k += 1
                    # ex = c * exp(-sq/2)
                    nc.scalar.activation(ex[:, :w], sq[:, :w], Act.Exp,
                                         bias=lnc, scale=-0.5)
                    # gg = (sq - 1) * ex   (= -true gg; S1T is negated)
                    nc.vector.scalar_tensor_tensor(gg[:, q, a:b], sq[:, :w], 1.0,
                                                   ex[:, :w],
                                                   op0=Alu.subtract, op1=Alu.mult)
                # stage A: H[i,:] = sum_j gg[j,i] * S2[j,:] for chunks in [a,b)
                for t in range(a // 128, b // 128):
                    for (cc0, ccw) in col_chunks:
                        h_ps = hpool.tile([128, 512], F32, tag="hps")
                        for q in range(NJ2):
                            nc.tensor.matmul(
                                h_ps[:, :ccw],
                                gg[:, q, t * 128:(t + 1) * 128],
                                s2_sb[:, q, cc0:cc0 + ccw],
                                start=(q == 0), stop=(q == NJ2 - 1))
                        eng = nc.vector if cp_k % 2 == 0 else nc.scalar
                        if eng is nc.vector:
                            nc.vector.tensor_copy(h_sb[:, t, cc0:cc0 + ccw],
                                                  h_ps[:, :ccw])
                        else:
                            nc.scalar.copy(h_sb[:, t, cc0:cc0 + ccw], h_ps[:, :ccw])
                        cp_k += 1
                # stage B for every block fully covered now
                done_t = b // 128
                for blk in range(NB):
                    t0, nch = block_chunks[blk]
                    if blocks_emitted[blk] or t0 + nch > done_t:
                        continue
                    blocks_emitted[blk] = True
                    o_sb = obpool.tile([126, W3], F32, tag="osb")
                    for (cc0, ccw) in col_chunks:
                        o_ps = opool.tile([126, 512], F32, tag="ops")
                        for k in range(nch):
                            t = t0 + k
                            nc.tensor.matmul(o_ps[:, :ccw], s1_sb[:, t, :],
                                             h_sb[:, t, cc0:cc0 + ccw],
                                             start=(k == 0), stop=(k == nch - 1))
                        eng_v = cp_k % 2 == 0
                        if eng_v:
                            nc.vector.tensor_copy(o_sb[:, cc0:cc0 + ccw],
                                                  o_ps[:, :ccw])
                        else:
                            nc.scalar.copy(o_sb[:, cc0:cc0 + ccw], o_ps[:, :ccw])
                        cp_k += 1
                    nc.sync.dma_start(out=out_h[blk * 126:(blk + 1) * 126, :],
                                      in_=o_sb[:, :])
    nc.compile()
    return nc


def kernel(**inputs):
    global LAST_RESULTS
    d1 = np.asarray(inputs["d1"], dtype=np.float32).reshape(-1)
    u1 = np.asarray(inputs["u1"], dtype=np.float32)
    d2 = np.asarray(inputs["d2"], dtype=np.float32).reshape(-1)
    u2 = np.asarray(inputs["u2"], dtype=np.float32)
    ls = float(np.asarray(inputs["lengthscale"]).reshape(-1)[0])
    i1 = np.asarray(inputs["i1"]).reshape(-1).astype(np.int64)
    i2 = np.asarray(inputs["i2"]).reshape(-1).astype(np.int64)
    na1 = int(np.asarray(inputs["natoms1"]))
    na2 = int(np.asarray(inputs["natoms2"]))
    n1 = d1.shape[0]
    n2 = d2.shape[0]

    c = 1.0 / (ls * ls)
    sqrtc = math.sqrt(c)
    lnc = math.log(c)

    # ---- axis 1: sort by i1, pack into 42-atom blocks padded to 128 ----
    o1 = np.argsort(i1, kind="stable")
    d1s, u1s, i1s = d1[o1], u1[o1], i1[o1]
    nb = (na1 + ABLK - 1) // ABLK
    bnd = np.searchsorted(i1s, np.arange(nb + 1) * ABLK)
    bnd[-1] = n1
    block_chunks = []
    segs_d1 = []
    segs_s1 = []
    coff = 0
    for blk in range(nb):
        st, en = int(bnd[blk]), int(bnd[blk + 1])
        cnt = en - st
        nch = max(1, (cnt + 127) // 128)
        ip = nch * 128
        dseg = np.zeros(ip, np.float32)
        dseg[:cnt] = d1s[st:en]
        sseg = np.zeros((ip, 126), np.float32)
        if cnt:
            rows = np.arange(cnt)
            loc = (i1s[st:en] - blk * ABLK).astype(np.int64)
            for a in range(3):
                sseg[rows, 3 * loc + a] = -u1s[st:en, a]  # negated: sign trick
        segs_d1.append(dseg)
        segs_s1.append(sseg)
        block_chunks.append((coff, nch))
        coff += nch
    d1p = np.concatenate(segs_d1)
    s1t = np.concatenate(segs_s1).reshape(-1, 128, 126)
    IPAD = d1p.shape[0]
    block_chunks = tuple(block_chunks)

    # ---- axis 2: sort by i2, shard uniformly across cores ----
    o2 = np.argsort(i2, kind="stable")
    d2s, u2s, i2s = d2[o2], u2[o2], i2[o2]
    npc = (n2 + NCORES - 1) // NCORES
    P2 = max(1, (npc + 127) // 128) * 128
    NJ2 = P2 // 128
    lo = np.zeros(NCORES, np.int64)
    width = np.zeros(NCORES, np.int64)
    core_slices = []
    for cc in range(NCORES):
        st = cc * npc
        en = min(n2, st + npc)
        core_slices.append((st, en))
        if en > st:
            lo[cc] = i2s[st]
            width[cc] = i2s[en - 1] - i2s[st] + 1
    W = int(width.max()) if n2 else 1
    W3 = 3 * W

    nc = _PROGRAM_CACHE.get((IPAD, NJ2, W3, block_chunks, sqrtc, lnc))
    if nc is None:
        nc = _build_program(IPAD, NJ2, W3, block_chunks, sqrtc, lnc)
        _PROGRAM_CACHE[(IPAD, NJ2, W3, block_chunks, sqrtc, lnc)] = nc

    in_maps = []
    for cc in range(NCORES):
        st, en = core_slices[cc]
        cnt = en - st
        d2col = np.zeros((NJ2, 128), np.float32)
        d2col.reshape(-1)[:cnt] = sqrtc * d2s[st:en]
        s2 = np.zeros((P2, W3), np.float32)
        if cnt:
            rows = np.arange(cnt)
            loc = (i2s[st:en] - lo[cc]).astype(np.int64)
            for b in range(3):
                s2[rows, 3 * loc + b] = u2s[st:en, b]
        in_maps.append({
            "d1p": d1p,
            "d2col": np.ascontiguousarray(d2col.T),
            "s2": s2.reshape(NJ2, 128, W3),
            "s1t": s1t,
        })

    from concourse.bass_utils import run_bass_kernel_spmd
    if TRACE:
        _install_ntff_hook()
    res = run_bass_kernel_spmd(nc, in_maps, core_ids=list(range(NCORES)),
                               trace=TRACE)
    LAST_RESULTS = res

    out = np.zeros((3 * na1, 3 * na2), np.float32)
    for cc in range(NCORES):
        st, en = core_slices[cc]
        if en <= st:
            continue
        w3 = 3 * int(width[cc])
        col0 = 3 * int(lo[cc])
        out[:, col0:col0 + w3] += res.results[cc]["out"][:3 * na1, :w3]
    return out
